# revision 18
# baseline (speedup 1.0000x reference)
"""Causal self-attention (B=2, T=2048, C=1024, H=16, D=64) on 8 TRN2 NeuronCores.

Sharding (head-parallel with on-device collectives, bf16 transfers):
  Core c owns heads (2c, 2c+1) for BOTH batches.
  Host ships per core (all bf16 unless noted):
    xt      [128, 4096]  x^T feature rows 128c..128c+128, cols = [b0 | b1] tokens
    w_qkv   [1024, 384]  W_qkv columns for its 2 heads, ordered [q | k | v]
    b_qk    [128, 2]     f32 biases for the q,k row-blocks
    b_v     [1, 128]     bf16 v bias row
    w_proj  [1024, 128]  W_proj column slice 128c..128c+128
  Device:
    AllGather x^T -> [1024, 4096] (per batch halves, overlap with compute)
    qk^T = (x W_qk + b)^T  [256, 2048] per batch; v = x W_v + b  [2048, 128]
    per (batch, head): chunked causal softmax(q k^T / 8) @ v with ones-append
      row-sum trick -> y^T [64, 2048]
    AllGather y^T over cores -> [1024, 2048] per batch (full head dim)
    out^T slice = w_proj_slice^T @ y^T_full  [128, 4096] bf16 -> host
  Host: assemble out columns, + b_proj, cast f32.
"""

import sys

if "/opt/trn_rl_repo" not in sys.path:
    sys.path.insert(0, "/opt/trn_rl_repo")

from contextlib import ExitStack

import numpy as np

import concourse.bacc as bacc
import concourse.mybir as mybir
import concourse.tile as tile
from concourse.masks import make_upper_triangular

N_CORES = 8
B = 2
T = 2048          # tokens per batch
TT = B * T        # 4096
C = 1024
HL = 2            # local heads per core
D = 64            # head dim
QK = 2 * HL * D   # 256 q+k channels per core
V = HL * D        # 128 v channels per core
P = 128
NT = T // P       # 16 token tiles per batch
NCC = C // P      # 8 contraction chunks
SCALE = D ** -0.5
f32 = mybir.dt.float32
bf16 = mybir.dt.bfloat16
AF = mybir.ActivationFunctionType
RG = [list(range(N_CORES))]


def _slices_aligned(start, end):
    """[start, end) split on the 512 grid (PSUM-bank-aligned outputs)."""
    out = []
    n0 = start
    while n0 < end:
        n1 = min(end, (n0 // 512 + 1) * 512)
        out.append((n0, n1))
        n0 = n1
    return out


def build():
    nc = bacc.Bacc("TRN2", target_bir_lowering=False, debug=False,
                   num_devices=N_CORES)

    xt_ap = nc.dram_tensor("xt", [P, TT], bf16, kind="ExternalInput").ap()
    w_qkv_ap = nc.dram_tensor("w_qkv", [C, 3 * V], bf16,
                              kind="ExternalInput").ap()
    b_qk_ap = nc.dram_tensor("b_qk", [P, 2], f32, kind="ExternalInput").ap()
    b_v_ap = nc.dram_tensor("b_v", [1, V], bf16, kind="ExternalInput").ap()
    w_proj_ap = nc.dram_tensor("w_proj", [C, P], bf16,
                               kind="ExternalInput").ap()
    out_ap = nc.dram_tensor("outT", [P, TT], bf16, kind="ExternalOutput").ap()

    with tile.TileContext(nc) as tc, ExitStack() as ctx:
        dram = ctx.enter_context(tc.tile_pool(name="dram", bufs=1,
                                              space="DRAM"))
        agx_in = [dram.tile([P, T], bf16, tag=f"agxi{b}", name=f"agxi{b}")
                  for b in range(B)]
        agx_out = [dram.tile([C, T], bf16, tag=f"agxo{b}", name=f"agxo{b}",
                             addr_space="Shared")
                   for b in range(B)]
        agy_in = [dram.tile([P, T], bf16, tag=f"agyi{b}", name=f"agyi{b}")
                  for b in range(B)]
        agy_out = [dram.tile([C, T], bf16, tag=f"agyo{b}", name=f"agyo{b}",
                             addr_space="Shared")
                   for b in range(B)]

        # stage x^T halves into collective bounce buffers, kick AllGathers
        for b in range(B):
            nc.gpsimd.dma_start(agx_in[b][:], xt_ap[:, b * T:(b + 1) * T])
        for b in range(B):
            nc.gpsimd.collective_compute(
                "AllGather", mybir.AluOpType.bypass, replica_groups=RG,
                ins=[agx_in[b].opt()], outs=[agx_out[b].opt()])

        const_pool = ctx.enter_context(tc.tile_pool(name="const", bufs=1))
        # keep element [j, i] iff j <= i  (upper triangular incl diag)
        mask01 = const_pool.tile([P, P], bf16, tag="mask01", name="mask01")
        make_upper_triangular(nc, mask01[:], val=1.0, diag=True)
        ones_row = const_pool.tile([1, P], bf16, tag="ones", name="ones")
        nc.vector.memset(ones_row[:], 1.0)
        ones_f32 = const_pool.tile([1, D], f32, tag="onesf", name="onesf")
        nc.vector.memset(ones_f32[:], 1.0)
        ones_col = const_pool.tile([P, HL], bf16, tag="onesc", name="onesc")
        nc.vector.memset(ones_col[:], 1.0)
        bqk_t = const_pool.tile([P, 2], f32, tag="bqk", name="bqk")
        nc.sync.dma_start(bqk_t[:], b_qk_ap)
        bv_row = const_pool.tile([1, V], bf16, tag="bv", name="bv")
        nc.sync.dma_start(bv_row[:], b_v_ap)

        # weights resident in SBUF
        w_pool = ctx.enter_context(tc.tile_pool(name="wp", bufs=1))
        wqkv = [w_pool.tile([P, 3 * V], bf16, tag=f"wqkv{c}", name=f"wqkv{c}")
                for c in range(NCC)]
        wproj = [w_pool.tile([P, P], bf16, tag=f"wpr{c}", name=f"wpr{c}")
                 for c in range(NCC)]
        wqkv_view = w_qkv_ap.rearrange("(c p) n -> c p n", p=P)
        wproj_view = w_proj_ap.rearrange("(c p) n -> c p n", p=P)
        for c in range(NCC):
            nc.sync.dma_start(wqkv[c][:], wqkv_view[c])
            nc.sync.dma_start(wproj[c][:], wproj_view[c])

        # persistent intermediates
        qk_pool = ctx.enter_context(tc.tile_pool(name="qkp", bufs=1))
        qk_sb = [qk_pool.tile([P, TT], bf16, tag=f"qk{m}", name=f"qk{m}")
                 for m in range(2)]
        v_pool = ctx.enter_context(tc.tile_pool(name="vp", bufs=1))
        v_sb = [v_pool.tile([P, HL * (D + 1)], bf16, tag=f"v{t}",
                            name=f"v{t}") for t in range(B * NT)]
        yT_pool = ctx.enter_context(tc.tile_pool(name="yTp", bufs=1))
        yT_sb = [yT_pool.tile([P, T], bf16, tag=f"yT{b}", name=f"yT{b}")
                 for b in range(B)]

        # ---------------- Phase A: qk^T and v per batch ----------------
        def phase_a(b):
            for ts in range(4):
                s0 = ts * 512
                xsb = []
                for c in range(NCC):
                    xt_t = xsb_pool.tile([P, 512], bf16, tag=f"x{c}",
                                         name="xt_t")
                    nc.sync.dma_start(
                        xt_t[:], agx_out[b][c * P:(c + 1) * P, s0:s0 + 512])
                    xsb.append(xt_t)
                for m in range(2):
                    ps = qkps_pool.tile([P, 512], f32, tag="qkps", name="ps")
                    for c in range(NCC):
                        nc.tensor.matmul(
                            ps[:], lhsT=wqkv[c][:, m * P:(m + 1) * P],
                            rhs=xsb[c][:], start=(c == 0), stop=(c == NCC - 1))
                    nc.scalar.activation(
                        qk_sb[m][:, b * T + s0:b * T + s0 + 512], ps[:],
                        AF.Identity, bias=bqk_t[:, m:m + 1], scale=1.0)
                for tt in range(4):
                    vp = vps_pool.tile([P, V], f32, tag="vps", name="vp")
                    for c in range(NCC):
                        nc.tensor.matmul(
                            vp[:], lhsT=xsb[c][:, tt * P:(tt + 1) * P],
                            rhs=wqkv[c][:, 2 * P:3 * P],
                            start=(c == 0), stop=False)
                    # bias as rank-1 update: ones[T,1] @ b_v[1,V]
                    nc.tensor.matmul(
                        vp[:], lhsT=ones_row[0:1, 0:P], rhs=bv_row[:],
                        start=False, stop=True)
                    v3 = v_sb[b * NT + ts * 4 + tt][:].rearrange(
                        "p (h e) -> p h e", e=D + 1)
                    nc.vector.tensor_copy(
                        v3[:, :, 0:D],
                        vp[:].rearrange("p (h d) -> p h d", d=D))
                    nc.vector.tensor_copy(
                        v3[:, :, D:D + 1],
                        ones_col[:].rearrange("p (h o) -> p h o", o=1))

        # ---------------- Phase B: attention per (batch, head) ----------
        def phase_b(b, h):
            po = h * D
            qT = qk_sb[0][po:po + D, b * T:(b + 1) * T]
            kT = qk_sb[1][po:po + D, b * T:(b + 1) * T]
            yext = yext_pool.tile([D + 1, T], f32, tag="yext", name="yext")

            def emit_st_exp(c):
                """s^T matmuls + exp for chunk c -> pT tile."""
                q0 = c * P
                pT = pt_pool.tile([P, T], bf16, tag="pt", name="pT")
                for (n0, n1) in _slices_aligned(q0, T):
                    sp = sps_pool.tile([P, n1 - n0], f32, tag="sps",
                                       name="sp")
                    nc.tensor.matmul(
                        sp[:], lhsT=kT[:, q0:q0 + P], rhs=qT[:, n0:n1],
                        start=True, stop=True)
                    nc.scalar.activation(
                        pT[:, n0:n1], sp[:], AF.Exp, bias=0.0, scale=SCALE)
                # causal mask inside the diagonal block
                nc.vector.tensor_mul(
                    pT[:, q0:q0 + P], pT[:, q0:q0 + P], mask01[:])
                return pT

            def emit_pv(c, pT):
                q0 = c * P
                for (n0, n1) in _slices_aligned(q0, T):
                    nc.tensor.matmul(
                        yext[:, n0:n1],
                        lhsT=v_sb[b * NT + c][:, h * (D + 1):(h + 1) * (D + 1)],
                        rhs=pT[:, n0:n1],
                        start=(c == 0), stop=(c == NT - 1),
                        skip_group_check=True)

            # software pipeline: emit s^T(c+1) before pv(c) so the PE
            # fills the exp(c) latency with the next chunk's matmuls
            pT_prev = emit_st_exp(0)
            for c in range(1, NT):
                pT_cur = emit_st_exp(c)
                emit_pv(c - 1, pT_prev)
                pT_prev = pT_cur
            emit_pv(NT - 1, pT_prev)
            # normalize rows by l (last partition row of yext) and
            # store into yT in [d, T] layout
            for g2 in range(4):
                s0, s1 = g2 * 512, (g2 + 1) * 512
                rr = rr_pool.tile([1, 512], f32, tag="rr", name="rr")
                nc.vector.reciprocal(rr[:], yext[D:D + 1, s0:s1])
                bp = bps_pool.tile([D, 512], f32, tag="bp", name="bp")
                nc.tensor.matmul(bp[:], lhsT=ones_f32[:], rhs=rr[:],
                                 start=True, stop=True)
                rb = rbc_pool.tile([D, 512], f32, tag="rbc", name="rb")
                nc.vector.tensor_copy(rb[:], bp[:])
                with nc.allow_low_precision(
                        reason="bf16 y store; 2e-2 rel-err budget"):
                    nc.vector.tensor_mul(
                        yT_sb[b][po:po + D, s0:s1], yext[0:D, s0:s1], rb[:])

        # ---------------- Phase C: output projection per batch ----------
        def launch_agy(b):
            nc.gpsimd.dma_start(agy_in[b][:], yT_sb[b][:])
            nc.gpsimd.collective_compute(
                "AllGather", mybir.AluOpType.bypass, replica_groups=RG,
                ins=[agy_in[b].opt()], outs=[agy_out[b].opt()])

        def phase_c(b):
            for ts in range(4):
                s0 = ts * 512
                pp = pp_pool.tile([P, 512], f32, tag="pp", name="pp")
                for c in range(NCC):
                    ysb = ysb_pool.tile([P, 512], bf16, tag="y", name="ysb")
                    nc.sync.dma_start(
                        ysb[:], agy_out[b][c * P:(c + 1) * P, s0:s0 + 512])
                    nc.tensor.matmul(pp[:], lhsT=wproj[c][:], rhs=ysb[:],
                                     start=(c == 0), stop=(c == NCC - 1))
                ob = osb_pool.tile([P, 512], bf16, tag="osb", name="ob")
                nc.scalar.copy(ob[:], pp[:])
                nc.sync.dma_start(out_ap[:, b * T + s0:b * T + s0 + 512],
                                  ob[:])

        with ExitStack() as actx:
            xsb_pool = actx.enter_context(tc.tile_pool(name="xsb", bufs=2))
            qkps_pool = actx.enter_context(
                tc.tile_pool(name="qkps", bufs=3, space="PSUM"))
            vps_pool = actx.enter_context(
                tc.tile_pool(name="vps", bufs=4, space="PSUM"))
            phase_a(0)
            phase_a(1)
        with ExitStack() as bctx:
            pt_pool = bctx.enter_context(tc.tile_pool(name="pt", bufs=3))
            rr_pool = bctx.enter_context(tc.tile_pool(name="rr", bufs=2))
            rbc_pool = bctx.enter_context(tc.tile_pool(name="rbc", bufs=2))
            sps_pool = bctx.enter_context(
                tc.tile_pool(name="sps", bufs=3, space="PSUM"))
            bps_pool = bctx.enter_context(
                tc.tile_pool(name="bps", bufs=1, space="PSUM"))
            yext_pool = bctx.enter_context(
                tc.tile_pool(name="yext", bufs=1, space="PSUM"))
            phase_b(0, 0)
            phase_b(0, 1)
            launch_agy(0)      # overlaps batch-1 attention
            phase_b(1, 0)
            phase_b(1, 1)
            launch_agy(1)
        ysb_pool = ctx.enter_context(tc.tile_pool(name="ysb", bufs=4))
        osb_pool = ctx.enter_context(tc.tile_pool(name="osb", bufs=3))
        pp_pool = ctx.enter_context(
            tc.tile_pool(name="pp", bufs=2, space="PSUM"))
        phase_c(0)             # overlaps agy(1)
        phase_c(1)

    nc.compile()
    return nc


_NC = None


def _get_nc():
    global _NC
    if _NC is None:
        _NC = build()
    return _NC


def make_in_maps(x, W_qkv, b_qkv, W_proj):
    """Per-core input dicts (host-side sharding, bf16)."""
    import ml_dtypes

    bf = ml_dtypes.bfloat16
    x = np.asarray(x, dtype=np.float32)
    W_qkv = np.asarray(W_qkv, dtype=np.float32)
    b_qkv = np.asarray(b_qkv, dtype=np.float32)
    W_proj = np.asarray(W_proj, dtype=np.float32)
    # x^T with both batches side by side: [C, B*T]
    xT = np.concatenate([x[0].T, x[1].T], axis=1).astype(bf)
    Wb = W_qkv.astype(bf)
    Wpb = W_proj.astype(bf)
    in_maps = []
    for c in range(N_CORES):
        q0 = P * c
        k0 = C + P * c
        v0 = 2 * C + P * c
        in_maps.append({
            "xt": np.ascontiguousarray(xT[P * c:P * (c + 1), :]),
            "w_qkv": np.ascontiguousarray(
                np.concatenate([Wb[:, q0:q0 + P], Wb[:, k0:k0 + P],
                                Wb[:, v0:v0 + P]], axis=1)),
            "b_qk": np.ascontiguousarray(
                np.stack([b_qkv[q0:q0 + P], b_qkv[k0:k0 + P]], axis=1)),
            "b_v": np.ascontiguousarray(
                b_qkv[v0:v0 + P].astype(bf).reshape(1, V)),
            "w_proj": np.ascontiguousarray(Wpb[:, P * c:P * (c + 1)]),
        })
    return in_maps


def combine(results, b_proj):
    """Host-side unshard: concat per-core out^T column slices, + bias."""
    b_proj = np.asarray(b_proj, dtype=np.float32)
    # results[c]["outT"]: [128, 4096] bf16 -> out[:, :, 128c:128c+128]
    cols = [results[c]["outT"].astype(np.float32) for c in range(N_CORES)]
    full = np.concatenate(cols, axis=0)            # [1024, 4096]
    out = full.T.reshape(B, T, C) + b_proj
    return np.ascontiguousarray(out)


def kernel(x, W_qkv, b_qkv, W_proj, b_proj):
    from concourse.bass_utils import run_bass_kernel_spmd

    nc = _get_nc()
    in_maps = make_in_maps(x, W_qkv, b_qkv, W_proj)
    res = run_bass_kernel_spmd(nc, in_maps, list(range(N_CORES)))
    return combine(res.results, b_proj)


# revision 22
# speedup vs baseline: 1.0305x; 1.0305x over previous
"""Causal self-attention (B=2, T=2048, C=1024, H=16, D=64) on 8 TRN2 NeuronCores.

Sharding (head-parallel with on-device collectives, bf16 transfers):
  Core c owns heads (2c, 2c+1) for BOTH batches.
  Host ships per core (all bf16 unless noted):
    xt      [128, 4096]  x^T feature rows 128c..128c+128, cols = [b0 | b1] tokens
    w_qkv   [1024, 384]  W_qkv columns for its 2 heads, ordered [q | k | v]
    b_qk    [128, 2]     f32 biases for the q,k row-blocks
    b_v     [1, 128]     bf16 v bias row
    w_proj  [1024, 128]  W_proj column slice 128c..128c+128
  Device:
    AllGather x^T -> [1024, 4096] (per batch halves, overlap with compute)
    qk^T = (x W_qk + b)^T  [256, 2048] per batch; v = x W_v + b  [2048, 128]
    per (batch, head): chunked causal softmax(q k^T / 8) @ v with ones-append
      row-sum trick -> y^T [64, 2048]
    AllGather y^T over cores -> [1024, 2048] per batch (full head dim)
    out^T slice = w_proj_slice^T @ y^T_full  [128, 4096] bf16 -> host
  Host: assemble out columns, + b_proj, cast f32.
"""

import sys

if "/opt/trn_rl_repo" not in sys.path:
    sys.path.insert(0, "/opt/trn_rl_repo")

from contextlib import ExitStack

import numpy as np

import concourse.bacc as bacc
import concourse.mybir as mybir
import concourse.tile as tile
from concourse.masks import make_upper_triangular

N_CORES = 8
B = 2
T = 2048          # tokens per batch
TT = B * T        # 4096
C = 1024
HL = 2            # local heads per core
D = 64            # head dim
QK = 2 * HL * D   # 256 q+k channels per core
V = HL * D        # 128 v channels per core
P = 128
NT = T // P       # 16 token tiles per batch
NCC = C // P      # 8 contraction chunks
SCALE = D ** -0.5
f32 = mybir.dt.float32
bf16 = mybir.dt.bfloat16
AF = mybir.ActivationFunctionType
RG = [list(range(N_CORES))]


def _slices_aligned(start, end):
    """[start, end) split on the 512 grid (PSUM-bank-aligned outputs)."""
    out = []
    n0 = start
    while n0 < end:
        n1 = min(end, (n0 // 512 + 1) * 512)
        out.append((n0, n1))
        n0 = n1
    return out


def build():
    nc = bacc.Bacc("TRN2", target_bir_lowering=False, debug=False,
                   num_devices=N_CORES)

    xt_ap = nc.dram_tensor("xt", [P, TT], bf16, kind="ExternalInput").ap()
    w_qkv_ap = nc.dram_tensor("w_qkv", [C, 3 * V], bf16,
                              kind="ExternalInput").ap()
    b_qk_ap = nc.dram_tensor("b_qk", [P, 2], f32, kind="ExternalInput").ap()
    b_v_ap = nc.dram_tensor("b_v", [1, V], bf16, kind="ExternalInput").ap()
    w_proj_ap = nc.dram_tensor("w_proj", [C, P], bf16,
                               kind="ExternalInput").ap()
    out_ap = nc.dram_tensor("outT", [P, TT], bf16, kind="ExternalOutput").ap()

    with tile.TileContext(nc) as tc, ExitStack() as ctx:
        dram = ctx.enter_context(tc.tile_pool(name="dram", bufs=1,
                                              space="DRAM"))
        agx_in = [dram.tile([P, T], bf16, tag=f"agxi{b}", name=f"agxi{b}")
                  for b in range(B)]
        agx_out = [dram.tile([C, T], bf16, tag=f"agxo{b}", name=f"agxo{b}",
                             addr_space="Shared")
                   for b in range(B)]
        agy_in = [dram.tile([P, T], bf16, tag=f"agyi{b}", name=f"agyi{b}")
                  for b in range(B)]
        agy_out = [dram.tile([C, T], bf16, tag=f"agyo{b}", name=f"agyo{b}",
                             addr_space="Shared")
                   for b in range(B)]

        # stage x^T halves into collective bounce buffers, kick AllGathers
        for b in range(B):
            nc.gpsimd.dma_start(agx_in[b][:], xt_ap[:, b * T:(b + 1) * T])
        for b in range(B):
            nc.gpsimd.collective_compute(
                "AllGather", mybir.AluOpType.bypass, replica_groups=RG,
                ins=[agx_in[b].opt()], outs=[agx_out[b].opt()])

        const_pool = ctx.enter_context(tc.tile_pool(name="const", bufs=1))
        # keep element [j, i] iff j <= i  (upper triangular incl diag)
        mask01 = const_pool.tile([P, P], bf16, tag="mask01", name="mask01")
        make_upper_triangular(nc, mask01[:], val=1.0, diag=True)
        ones_row = const_pool.tile([1, P], bf16, tag="ones", name="ones")
        nc.vector.memset(ones_row[:], 1.0)
        ones_f32 = const_pool.tile([1, D], f32, tag="onesf", name="onesf")
        nc.vector.memset(ones_f32[:], 1.0)
        ones_col = const_pool.tile([P, HL], bf16, tag="onesc", name="onesc")
        nc.vector.memset(ones_col[:], 1.0)
        bqk_t = const_pool.tile([P, 2], f32, tag="bqk", name="bqk")
        nc.sync.dma_start(bqk_t[:], b_qk_ap)
        bv_row = const_pool.tile([1, V], bf16, tag="bv", name="bv")
        nc.sync.dma_start(bv_row[:], b_v_ap)

        # weights resident in SBUF
        w_pool = ctx.enter_context(tc.tile_pool(name="wp", bufs=1))
        wqkv = [w_pool.tile([P, 3 * V], bf16, tag=f"wqkv{c}", name=f"wqkv{c}")
                for c in range(NCC)]
        wproj = [w_pool.tile([P, P], bf16, tag=f"wpr{c}", name=f"wpr{c}")
                 for c in range(NCC)]
        wqkv_view = w_qkv_ap.rearrange("(c p) n -> c p n", p=P)
        wproj_view = w_proj_ap.rearrange("(c p) n -> c p n", p=P)
        for c in range(NCC):
            nc.sync.dma_start(wqkv[c][:], wqkv_view[c])
            nc.sync.dma_start(wproj[c][:], wproj_view[c])

        # persistent intermediates
        qk_pool = ctx.enter_context(tc.tile_pool(name="qkp", bufs=1))
        qk_sb = [qk_pool.tile([P, TT], bf16, tag=f"qk{m}", name=f"qk{m}")
                 for m in range(2)]
        v_pool = ctx.enter_context(tc.tile_pool(name="vp", bufs=1))
        v_sb = [v_pool.tile([P, HL * (D + 1)], bf16, tag=f"v{t}",
                            name=f"v{t}") for t in range(B * NT)]
        yT_pool = ctx.enter_context(tc.tile_pool(name="yTp", bufs=1))
        yT_sb = [yT_pool.tile([P, T], bf16, tag=f"yT{b}", name=f"yT{b}")
                 for b in range(B)]

        # ---------------- Phase A: qk^T and v per batch ----------------
        def phase_a(b):
            for ts in range(4):
                s0 = ts * 512
                xsb = []
                for c in range(NCC):
                    xt_t = xsb_pool.tile([P, 512], bf16, tag=f"x{c}",
                                         name="xt_t")
                    nc.sync.dma_start(
                        xt_t[:], agx_out[b][c * P:(c + 1) * P, s0:s0 + 512])
                    xsb.append(xt_t)
                for m in range(2):
                    ps = qkps_pool.tile([P, 512], f32, tag="qkps", name="ps")
                    for c in range(NCC):
                        nc.tensor.matmul(
                            ps[:], lhsT=wqkv[c][:, m * P:(m + 1) * P],
                            rhs=xsb[c][:], start=(c == 0), stop=(c == NCC - 1))
                    nc.scalar.activation(
                        qk_sb[m][:, b * T + s0:b * T + s0 + 512], ps[:],
                        AF.Identity, bias=bqk_t[:, m:m + 1], scale=1.0)
                for tt in range(4):
                    vp = vps_pool.tile([P, V], f32, tag="vps", name="vp")
                    for c in range(NCC):
                        nc.tensor.matmul(
                            vp[:], lhsT=xsb[c][:, tt * P:(tt + 1) * P],
                            rhs=wqkv[c][:, 2 * P:3 * P],
                            start=(c == 0), stop=False)
                    # bias as rank-1 update: ones[T,1] @ b_v[1,V]
                    nc.tensor.matmul(
                        vp[:], lhsT=ones_row[0:1, 0:P], rhs=bv_row[:],
                        start=False, stop=True)
                    v3 = v_sb[b * NT + ts * 4 + tt][:].rearrange(
                        "p (h e) -> p h e", e=D + 1)
                    nc.vector.tensor_copy(
                        v3[:, :, 0:D],
                        vp[:].rearrange("p (h d) -> p h d", d=D))
                    nc.vector.tensor_copy(
                        v3[:, :, D:D + 1],
                        ones_col[:].rearrange("p (h o) -> p h o", o=1))

        # ---------------- Phase B: attention per (batch, head) ----------
        def phase_b(b, h):
            po = h * D
            qT = qk_sb[0][po:po + D, b * T:(b + 1) * T]
            kT = qk_sb[1][po:po + D, b * T:(b + 1) * T]
            yext = yext_pool.tile([D + 1, T], f32, tag="yext", name="yext")

            def emit_st_exp(c):
                """s^T matmuls + exp for chunk c -> pT tile.

                s psum tiles are [P, 1024] (2 banks): two N<=512 matmuls
                fill the halves, ONE exp activation covers both -- halves
                the ACT instruction count (185ns fixed cost each).
                """
                q0 = c * P
                pT = pt_pool.tile([P, T], bf16, tag="pt", name="pT")
                g0 = q0 // 1024
                for g in range(g0, 2):
                    b0, b1 = max(q0, g * 1024), (g + 1) * 1024
                    sp = sps_pool.tile([P, 1024], f32, tag="sps", name="sp")
                    off = 1024 - (b1 - b0)
                    for (n0, n1) in _slices_aligned(b0, b1):
                        nc.tensor.matmul(
                            sp[:, off + (n0 - b0):off + (n1 - b0)],
                            lhsT=kT[:, q0:q0 + P], rhs=qT[:, n0:n1],
                            start=True, stop=True)
                    nc.scalar.activation(
                        pT[:, b0:b1], sp[:, off:1024], AF.Exp,
                        bias=0.0, scale=SCALE)
                # causal mask inside the diagonal block
                nc.vector.tensor_mul(
                    pT[:, q0:q0 + P], pT[:, q0:q0 + P], mask01[:])
                return pT

            def emit_pv(c, pT):
                q0 = c * P
                for (n0, n1) in _slices_aligned(q0, T):
                    nc.tensor.matmul(
                        yext[:, n0:n1],
                        lhsT=v_sb[b * NT + c][:, h * (D + 1):(h + 1) * (D + 1)],
                        rhs=pT[:, n0:n1],
                        start=(c == 0), stop=(c == NT - 1),
                        skip_group_check=True)

            # software pipeline: emit s^T(c+1) before pv(c) so the PE
            # fills the exp(c) latency with the next chunk's matmuls
            pT_prev = emit_st_exp(0)
            for c in range(1, NT):
                pT_cur = emit_st_exp(c)
                emit_pv(c - 1, pT_prev)
                pT_prev = pT_cur
            emit_pv(NT - 1, pT_prev)
            # normalize rows by l (last partition row of yext) and
            # store into yT in [d, T] layout
            for g2 in range(4):
                s0, s1 = g2 * 512, (g2 + 1) * 512
                rr = rr_pool.tile([1, 512], f32, tag="rr", name="rr")
                nc.vector.reciprocal(rr[:], yext[D:D + 1, s0:s1])
                bpt = sps_pool.tile([P, 1024], f32, tag="sps", name="bp")
                bp = bpt[0:D, 0:512]
                nc.tensor.matmul(bp, lhsT=ones_f32[:], rhs=rr[:],
                                 start=True, stop=True)
                rb = rbc_pool.tile([D, 512], f32, tag="rbc", name="rb")
                nc.vector.tensor_copy(rb[:], bp)
                with nc.allow_low_precision(
                        reason="bf16 y store; 2e-2 rel-err budget"):
                    nc.vector.tensor_mul(
                        yT_sb[b][po:po + D, s0:s1], yext[0:D, s0:s1], rb[:])

        # ---------------- Phase C: output projection per batch ----------
        def launch_agy(b):
            nc.gpsimd.dma_start(agy_in[b][:], yT_sb[b][:])
            nc.gpsimd.collective_compute(
                "AllGather", mybir.AluOpType.bypass, replica_groups=RG,
                ins=[agy_in[b].opt()], outs=[agy_out[b].opt()])

        def phase_c(b):
            for ts in range(4):
                s0 = ts * 512
                pp = pp_pool.tile([P, 512], f32, tag="pp", name="pp")
                for c in range(NCC):
                    ysb = ysb_pool.tile([P, 512], bf16, tag="y", name="ysb")
                    nc.sync.dma_start(
                        ysb[:], agy_out[b][c * P:(c + 1) * P, s0:s0 + 512])
                    nc.tensor.matmul(pp[:], lhsT=wproj[c][:], rhs=ysb[:],
                                     start=(c == 0), stop=(c == NCC - 1))
                ob = osb_pool.tile([P, 512], bf16, tag="osb", name="ob")
                nc.scalar.copy(ob[:], pp[:])
                nc.sync.dma_start(out_ap[:, b * T + s0:b * T + s0 + 512],
                                  ob[:])

        with ExitStack() as actx:
            xsb_pool = actx.enter_context(tc.tile_pool(name="xsb", bufs=2))
            qkps_pool = actx.enter_context(
                tc.tile_pool(name="qkps", bufs=3, space="PSUM"))
            vps_pool = actx.enter_context(
                tc.tile_pool(name="vps", bufs=4, space="PSUM"))
            phase_a(0)
            phase_a(1)
        with ExitStack() as bctx:
            pt_pool = bctx.enter_context(tc.tile_pool(name="pt", bufs=3))
            rr_pool = bctx.enter_context(tc.tile_pool(name="rr", bufs=2))
            rbc_pool = bctx.enter_context(tc.tile_pool(name="rbc", bufs=2))
            sps_pool = bctx.enter_context(
                tc.tile_pool(name="sps", bufs=2, space="PSUM"))
            yext_pool = bctx.enter_context(
                tc.tile_pool(name="yext", bufs=1, space="PSUM"))
            phase_b(0, 0)
            phase_b(0, 1)
            launch_agy(0)      # overlaps batch-1 attention
            phase_b(1, 0)
            phase_b(1, 1)
            launch_agy(1)
        ysb_pool = ctx.enter_context(tc.tile_pool(name="ysb", bufs=4))
        osb_pool = ctx.enter_context(tc.tile_pool(name="osb", bufs=3))
        pp_pool = ctx.enter_context(
            tc.tile_pool(name="pp", bufs=2, space="PSUM"))
        phase_c(0)             # overlaps agy(1)
        phase_c(1)

    nc.compile()
    return nc


_NC = None


def _get_nc():
    global _NC
    if _NC is None:
        _NC = build()
    return _NC


def make_in_maps(x, W_qkv, b_qkv, W_proj):
    """Per-core input dicts (host-side sharding, bf16)."""
    import ml_dtypes

    bf = ml_dtypes.bfloat16
    x = np.asarray(x, dtype=np.float32)
    W_qkv = np.asarray(W_qkv, dtype=np.float32)
    b_qkv = np.asarray(b_qkv, dtype=np.float32)
    W_proj = np.asarray(W_proj, dtype=np.float32)
    # x^T with both batches side by side: [C, B*T]
    xT = np.concatenate([x[0].T, x[1].T], axis=1).astype(bf)
    Wb = W_qkv.astype(bf)
    Wpb = W_proj.astype(bf)
    in_maps = []
    for c in range(N_CORES):
        q0 = P * c
        k0 = C + P * c
        v0 = 2 * C + P * c
        in_maps.append({
            "xt": np.ascontiguousarray(xT[P * c:P * (c + 1), :]),
            "w_qkv": np.ascontiguousarray(
                np.concatenate([Wb[:, q0:q0 + P], Wb[:, k0:k0 + P],
                                Wb[:, v0:v0 + P]], axis=1)),
            "b_qk": np.ascontiguousarray(
                np.stack([b_qkv[q0:q0 + P], b_qkv[k0:k0 + P]], axis=1)),
            "b_v": np.ascontiguousarray(
                b_qkv[v0:v0 + P].astype(bf).reshape(1, V)),
            "w_proj": np.ascontiguousarray(Wpb[:, P * c:P * (c + 1)]),
        })
    return in_maps


def combine(results, b_proj):
    """Host-side unshard: concat per-core out^T column slices, + bias."""
    b_proj = np.asarray(b_proj, dtype=np.float32)
    # results[c]["outT"]: [128, 4096] bf16 -> out[:, :, 128c:128c+128]
    cols = [results[c]["outT"].astype(np.float32) for c in range(N_CORES)]
    full = np.concatenate(cols, axis=0)            # [1024, 4096]
    out = full.T.reshape(B, T, C) + b_proj
    return np.ascontiguousarray(out)


def kernel(x, W_qkv, b_qkv, W_proj, b_proj):
    from concourse.bass_utils import run_bass_kernel_spmd

    nc = _get_nc()
    in_maps = make_in_maps(x, W_qkv, b_qkv, W_proj)
    res = run_bass_kernel_spmd(nc, in_maps, list(range(N_CORES)))
    return combine(res.results, b_proj)


# revision 24
# speedup vs baseline: 1.0328x; 1.0022x over previous
"""Causal self-attention (B=2, T=2048, C=1024, H=16, D=64) on 8 TRN2 NeuronCores.

Sharding (head-parallel with on-device collectives, bf16 transfers):
  Core c owns heads (2c, 2c+1) for BOTH batches.
  Host ships per core (all bf16 unless noted):
    xt      [128, 4096]  x^T feature rows 128c..128c+128, cols = [b0 | b1] tokens
    w_qkv   [1024, 384]  W_qkv columns for its 2 heads, ordered [q | k | v]
    b_qk    [128, 2]     f32 biases for the q,k row-blocks
    b_v     [1, 128]     bf16 v bias row
    w_proj  [1024, 128]  W_proj column slice 128c..128c+128
  Device:
    AllGather x^T -> [1024, 4096] (per batch halves, overlap with compute)
    qk^T = (x W_qk + b)^T  [256, 2048] per batch; v = x W_v + b  [2048, 128]
    per (batch, head): chunked causal softmax(q k^T / 8) @ v with ones-append
      row-sum trick -> y^T [64, 2048]
    AllGather y^T over cores -> [1024, 2048] per batch (full head dim)
    out^T slice = w_proj_slice^T @ y^T_full  [128, 4096] bf16 -> host
  Host: assemble out columns, + b_proj, cast f32.
"""

import sys

if "/opt/trn_rl_repo" not in sys.path:
    sys.path.insert(0, "/opt/trn_rl_repo")

from contextlib import ExitStack

import numpy as np

import concourse.bacc as bacc
import concourse.mybir as mybir
import concourse.tile as tile
from concourse.masks import make_upper_triangular

N_CORES = 8
B = 2
T = 2048          # tokens per batch
TT = B * T        # 4096
C = 1024
HL = 2            # local heads per core
D = 64            # head dim
QK = 2 * HL * D   # 256 q+k channels per core
V = HL * D        # 128 v channels per core
P = 128
NT = T // P       # 16 token tiles per batch
NCC = C // P      # 8 contraction chunks
SCALE = D ** -0.5
f32 = mybir.dt.float32
bf16 = mybir.dt.bfloat16
AF = mybir.ActivationFunctionType
RG = [list(range(N_CORES))]


def _slices_aligned(start, end):
    """[start, end) split on the 512 grid (PSUM-bank-aligned outputs)."""
    out = []
    n0 = start
    while n0 < end:
        n1 = min(end, (n0 // 512 + 1) * 512)
        out.append((n0, n1))
        n0 = n1
    return out


def build():
    nc = bacc.Bacc("TRN2", target_bir_lowering=False, debug=False,
                   num_devices=N_CORES)

    xt_ap = nc.dram_tensor("xt", [P, TT], bf16, kind="ExternalInput").ap()
    w_qkv_ap = nc.dram_tensor("w_qkv", [C, 3 * V], bf16,
                              kind="ExternalInput").ap()
    b_qk_ap = nc.dram_tensor("b_qk", [P, 2], f32, kind="ExternalInput").ap()
    b_v_ap = nc.dram_tensor("b_v", [1, V], bf16, kind="ExternalInput").ap()
    w_proj_ap = nc.dram_tensor("w_proj", [C, P], bf16,
                               kind="ExternalInput").ap()
    out_ap = nc.dram_tensor("outT", [P, TT], bf16, kind="ExternalOutput").ap()

    with tile.TileContext(nc) as tc, ExitStack() as ctx:
        dram = ctx.enter_context(tc.tile_pool(name="dram", bufs=1,
                                              space="DRAM"))
        agx_in = [dram.tile([P, T], bf16, tag=f"agxi{b}", name=f"agxi{b}")
                  for b in range(B)]
        agx_out = [dram.tile([C, T], bf16, tag=f"agxo{b}", name=f"agxo{b}",
                             addr_space="Shared")
                   for b in range(B)]
        agy_in = [dram.tile([P, T], bf16, tag=f"agyi{b}", name=f"agyi{b}")
                  for b in range(B)]
        agy_out = [dram.tile([C, T], bf16, tag=f"agyo{b}", name=f"agyo{b}",
                             addr_space="Shared")
                   for b in range(B)]

        # stage x^T halves into collective bounce buffers, kick AllGathers
        for b in range(B):
            nc.gpsimd.dma_start(agx_in[b][:], xt_ap[:, b * T:(b + 1) * T])
        for b in range(B):
            nc.gpsimd.collective_compute(
                "AllGather", mybir.AluOpType.bypass, replica_groups=RG,
                ins=[agx_in[b].opt()], outs=[agx_out[b].opt()])

        const_pool = ctx.enter_context(tc.tile_pool(name="const", bufs=1))
        # keep element [j, i] iff j <= i  (upper triangular incl diag)
        mask01 = const_pool.tile([P, P], bf16, tag="mask01", name="mask01")
        make_upper_triangular(nc, mask01[:], val=1.0, diag=True)
        ones_row = const_pool.tile([1, P], bf16, tag="ones", name="ones")
        nc.vector.memset(ones_row[:], 1.0)
        ones_f32 = const_pool.tile([1, D], f32, tag="onesf", name="onesf")
        nc.vector.memset(ones_f32[:], 1.0)
        ones_col = const_pool.tile([P, HL], bf16, tag="onesc", name="onesc")
        nc.vector.memset(ones_col[:], 1.0)
        bqk_t = const_pool.tile([P, 2], f32, tag="bqk", name="bqk")
        nc.sync.dma_start(bqk_t[:], b_qk_ap)
        bv_row = const_pool.tile([1, V], bf16, tag="bv", name="bv")
        nc.sync.dma_start(bv_row[:], b_v_ap)

        # weights resident in SBUF
        w_pool = ctx.enter_context(tc.tile_pool(name="wp", bufs=1))
        wqkv = [w_pool.tile([P, 3 * V], bf16, tag=f"wqkv{c}", name=f"wqkv{c}")
                for c in range(NCC)]
        wproj = [w_pool.tile([P, P], bf16, tag=f"wpr{c}", name=f"wpr{c}")
                 for c in range(NCC)]
        wqkv_view = w_qkv_ap.rearrange("(c p) n -> c p n", p=P)
        wproj_view = w_proj_ap.rearrange("(c p) n -> c p n", p=P)
        for c in range(NCC):
            nc.sync.dma_start(wqkv[c][:], wqkv_view[c])
            nc.sync.dma_start(wproj[c][:], wproj_view[c])

        # persistent intermediates
        qk_pool = ctx.enter_context(tc.tile_pool(name="qkp", bufs=1))
        qk_sb = [qk_pool.tile([P, TT], bf16, tag=f"qk{m}", name=f"qk{m}")
                 for m in range(2)]
        v_pool = ctx.enter_context(tc.tile_pool(name="vp", bufs=1))
        v_sb = [v_pool.tile([P, HL * (D + 1)], bf16, tag=f"v{t}",
                            name=f"v{t}") for t in range(B * NT)]
        yT_pool = ctx.enter_context(tc.tile_pool(name="yTp", bufs=1))
        yT_sb = [yT_pool.tile([P, T], bf16, tag=f"yT{b}", name=f"yT{b}")
                 for b in range(B)]

        # ---------------- Phase A: qk^T and v per batch ----------------
        def phase_a(b):
            for ts in range(4):
                s0 = ts * 512
                xsb = []
                for c in range(NCC):
                    xt_t = xsb_pool.tile([P, 512], bf16, tag=f"x{c}",
                                         name="xt_t")
                    nc.sync.dma_start(
                        xt_t[:], agx_out[b][c * P:(c + 1) * P, s0:s0 + 512])
                    xsb.append(xt_t)
                for m in range(2):
                    ps = qkps_pool.tile([P, 512], f32, tag="qkps", name="ps")
                    for c in range(NCC):
                        nc.tensor.matmul(
                            ps[:], lhsT=wqkv[c][:, m * P:(m + 1) * P],
                            rhs=xsb[c][:], start=(c == 0), stop=(c == NCC - 1))
                    nc.scalar.activation(
                        qk_sb[m][:, b * T + s0:b * T + s0 + 512], ps[:],
                        AF.Identity, bias=bqk_t[:, m:m + 1], scale=1.0)
                for tt in range(4):
                    vp = vps_pool.tile([P, V], f32, tag="vps", name="vp")
                    for c in range(NCC):
                        nc.tensor.matmul(
                            vp[:], lhsT=xsb[c][:, tt * P:(tt + 1) * P],
                            rhs=wqkv[c][:, 2 * P:3 * P],
                            start=(c == 0), stop=False)
                    # bias as rank-1 update: ones[T,1] @ b_v[1,V]
                    nc.tensor.matmul(
                        vp[:], lhsT=ones_row[0:1, 0:P], rhs=bv_row[:],
                        start=False, stop=True)
                    v3 = v_sb[b * NT + ts * 4 + tt][:].rearrange(
                        "p (h e) -> p h e", e=D + 1)
                    nc.vector.tensor_copy(
                        v3[:, :, 0:D],
                        vp[:].rearrange("p (h d) -> p h d", d=D))
                    nc.vector.tensor_copy(
                        v3[:, :, D:D + 1],
                        ones_col[:].rearrange("p (h o) -> p h o", o=1))

        # ---------------- Phase B: attention per batch, heads paired ----
        # The two local heads' s^T matmuls have K=64 contraction; head 0
        # lives on array rows 0-63 (base_partition 0), head 1 on rows
        # 64-127 (base_partition 64, auto tile_position) -- emitted
        # back-to-back they run CONCURRENTLY on the PE's row groups.
        # PSUM budget forces q-halving: per q-half, yext[h] is [65, 1024]
        # (2 banks) x 2 heads + 4 s-psum bufs = 8 banks.
        def phase_b_pair(b, qh):
            qbase = qh * 1024
            qend = qbase + 1024
            cmax = 8 * (qh + 1)
            qT = [qk_sb[0][h * D:(h + 1) * D, b * T:(b + 1) * T]
                  for h in range(HL)]
            kT = [qk_sb[1][h * D:(h + 1) * D, b * T:(b + 1) * T]
                  for h in range(HL)]
            yext = [yext_pool.tile([D + 1, 1024], f32, tag=f"yext{h}",
                                   name=f"yext{h}") for h in range(HL)]

            def emit_st_exp(c):
                """Paired s^T matmuls + exp for chunk c -> pT tiles."""
                k0 = c * P
                q0 = max(k0, qbase)
                pT = [pt_pool.tile([P, 1024], bf16, tag=f"pt{h}", name="pT")
                      for h in range(HL)]
                for (n0, n1) in _slices_aligned(q0, qend):
                    for h in range(HL):
                        sp = sps_pool.tile([P, 512], f32, tag="sps",
                                           name="sp")
                        nc.tensor.matmul(
                            sp[:, 0:n1 - n0], lhsT=kT[h][:, k0:k0 + P],
                            rhs=qT[h][:, n0:n1], start=True, stop=True)
                        nc.scalar.activation(
                            pT[h][:, n0 - qbase:n1 - qbase], sp[:, 0:n1 - n0],
                            AF.Exp, bias=0.0, scale=SCALE)
                if k0 >= qbase:
                    # causal mask inside the diagonal block
                    for h in range(HL):
                        nc.vector.tensor_mul(
                            pT[h][:, k0 - qbase:k0 - qbase + P],
                            pT[h][:, k0 - qbase:k0 - qbase + P], mask01[:])
                return pT

            def emit_pv(c, pT):
                k0 = c * P
                q0 = max(k0, qbase)
                for h in range(HL):
                    for (n0, n1) in _slices_aligned(q0, qend):
                        nc.tensor.matmul(
                            yext[h][:, n0 - qbase:n1 - qbase],
                            lhsT=v_sb[b * NT + c][:,
                                                  h * (D + 1):(h + 1) * (D + 1)],
                            rhs=pT[h][:, n0 - qbase:n1 - qbase],
                            start=(c == 0), stop=(c == cmax - 1),
                            skip_group_check=True)

            # software pipeline: emit s^T(c+1) before pv(c) so the PE
            # fills the exp(c) latency with the next chunk's matmuls
            pT_prev = emit_st_exp(0)
            for c in range(1, cmax):
                pT_cur = emit_st_exp(c)
                emit_pv(c - 1, pT_prev)
                pT_prev = pT_cur
            emit_pv(cmax - 1, pT_prev)
            # normalize rows by l (last partition row of yext) and
            # store into yT in [d, T] layout
            for h in range(HL):
                po = h * D
                for g2 in range(2):
                    s0, s1 = g2 * 512, (g2 + 1) * 512
                    rr = rr_pool.tile([1, 512], f32, tag="rr", name="rr")
                    nc.vector.reciprocal(rr[:], yext[h][D:D + 1, s0:s1])
                    bpt = sps_pool.tile([P, 512], f32, tag="sps", name="bp")
                    bp = bpt[0:D, :]
                    nc.tensor.matmul(bp, lhsT=ones_f32[:], rhs=rr[:],
                                     start=True, stop=True)
                    rb = rbc_pool.tile([D, 512], f32, tag="rbc", name="rb")
                    nc.vector.tensor_copy(rb[:], bp)
                    with nc.allow_low_precision(
                            reason="bf16 y store; 2e-2 rel-err budget"):
                        nc.vector.tensor_mul(
                            yT_sb[b][po:po + D, qbase + s0:qbase + s1],
                            yext[h][0:D, s0:s1], rb[:])

        # ---------------- Phase C: output projection per batch ----------
        def launch_agy(b):
            nc.gpsimd.dma_start(agy_in[b][:], yT_sb[b][:])
            nc.gpsimd.collective_compute(
                "AllGather", mybir.AluOpType.bypass, replica_groups=RG,
                ins=[agy_in[b].opt()], outs=[agy_out[b].opt()])

        def phase_c(b):
            for ts in range(4):
                s0 = ts * 512
                pp = pp_pool.tile([P, 512], f32, tag="pp", name="pp")
                for c in range(NCC):
                    ysb = ysb_pool.tile([P, 512], bf16, tag="y", name="ysb")
                    nc.sync.dma_start(
                        ysb[:], agy_out[b][c * P:(c + 1) * P, s0:s0 + 512])
                    nc.tensor.matmul(pp[:], lhsT=wproj[c][:], rhs=ysb[:],
                                     start=(c == 0), stop=(c == NCC - 1))
                ob = osb_pool.tile([P, 512], bf16, tag="osb", name="ob")
                nc.scalar.copy(ob[:], pp[:])
                nc.sync.dma_start(out_ap[:, b * T + s0:b * T + s0 + 512],
                                  ob[:])

        with ExitStack() as actx:
            xsb_pool = actx.enter_context(tc.tile_pool(name="xsb", bufs=2))
            qkps_pool = actx.enter_context(
                tc.tile_pool(name="qkps", bufs=3, space="PSUM"))
            vps_pool = actx.enter_context(
                tc.tile_pool(name="vps", bufs=4, space="PSUM"))
            phase_a(0)
            phase_a(1)
        with ExitStack() as bctx:
            pt_pool = bctx.enter_context(tc.tile_pool(name="pt", bufs=2))
            rr_pool = bctx.enter_context(tc.tile_pool(name="rr", bufs=2))
            rbc_pool = bctx.enter_context(tc.tile_pool(name="rbc", bufs=2))
            sps_pool = bctx.enter_context(
                tc.tile_pool(name="sps", bufs=4, space="PSUM"))
            yext_pool = bctx.enter_context(
                tc.tile_pool(name="yext", bufs=1, space="PSUM"))
            phase_b_pair(0, 0)
            phase_b_pair(0, 1)
            launch_agy(0)      # overlaps batch-1 attention
            phase_b_pair(1, 0)
            phase_b_pair(1, 1)
            launch_agy(1)
        ysb_pool = ctx.enter_context(tc.tile_pool(name="ysb", bufs=4))
        osb_pool = ctx.enter_context(tc.tile_pool(name="osb", bufs=3))
        pp_pool = ctx.enter_context(
            tc.tile_pool(name="pp", bufs=2, space="PSUM"))
        phase_c(0)             # overlaps agy(1)
        phase_c(1)

    nc.compile()
    return nc


_NC = None


def _get_nc():
    global _NC
    if _NC is None:
        _NC = build()
    return _NC


def make_in_maps(x, W_qkv, b_qkv, W_proj):
    """Per-core input dicts (host-side sharding, bf16)."""
    import ml_dtypes

    bf = ml_dtypes.bfloat16
    x = np.asarray(x, dtype=np.float32)
    W_qkv = np.asarray(W_qkv, dtype=np.float32)
    b_qkv = np.asarray(b_qkv, dtype=np.float32)
    W_proj = np.asarray(W_proj, dtype=np.float32)
    # x^T with both batches side by side: [C, B*T]
    xT = np.concatenate([x[0].T, x[1].T], axis=1).astype(bf)
    Wb = W_qkv.astype(bf)
    Wpb = W_proj.astype(bf)
    in_maps = []
    for c in range(N_CORES):
        q0 = P * c
        k0 = C + P * c
        v0 = 2 * C + P * c
        in_maps.append({
            "xt": np.ascontiguousarray(xT[P * c:P * (c + 1), :]),
            "w_qkv": np.ascontiguousarray(
                np.concatenate([Wb[:, q0:q0 + P], Wb[:, k0:k0 + P],
                                Wb[:, v0:v0 + P]], axis=1)),
            "b_qk": np.ascontiguousarray(
                np.stack([b_qkv[q0:q0 + P], b_qkv[k0:k0 + P]], axis=1)),
            "b_v": np.ascontiguousarray(
                b_qkv[v0:v0 + P].astype(bf).reshape(1, V)),
            "w_proj": np.ascontiguousarray(Wpb[:, P * c:P * (c + 1)]),
        })
    return in_maps


def combine(results, b_proj):
    """Host-side unshard: concat per-core out^T column slices, + bias."""
    b_proj = np.asarray(b_proj, dtype=np.float32)
    # results[c]["outT"]: [128, 4096] bf16 -> out[:, :, 128c:128c+128]
    cols = [results[c]["outT"].astype(np.float32) for c in range(N_CORES)]
    full = np.concatenate(cols, axis=0)            # [1024, 4096]
    out = full.T.reshape(B, T, C) + b_proj
    return np.ascontiguousarray(out)


def kernel(x, W_qkv, b_qkv, W_proj, b_proj):
    from concourse.bass_utils import run_bass_kernel_spmd

    nc = _get_nc()
    in_maps = make_in_maps(x, W_qkv, b_qkv, W_proj)
    res = run_bass_kernel_spmd(nc, in_maps, list(range(N_CORES)))
    return combine(res.results, b_proj)


# revision 30
# speedup vs baseline: 1.0467x; 1.0134x over previous
"""Causal self-attention (B=2, T=2048, C=1024, H=16, D=64) on 8 TRN2 NeuronCores.

Sharding (head-parallel with on-device collectives, bf16 transfers):
  Core c owns heads (2c, 2c+1) for BOTH batches.
  Host ships per core (all bf16 unless noted):
    xt      [128, 4096]  x^T feature rows 128c..128c+128, cols = [b0 | b1] tokens
    w_qkv   [1024, 384]  W_qkv columns for its 2 heads, ordered [q | k | v]
    b_qk    [128, 2]     f32 biases for the q,k row-blocks
    b_v     [1, 128]     bf16 v bias row
    w_proj  [1024, 128]  W_proj column slice 128c..128c+128
  Device:
    AllGather x^T -> [1024, 4096] (per batch halves, overlap with compute)
    qk^T = (x W_qk + b)^T  [256, 2048] per batch; v = x W_v + b  [2048, 128]
    per (batch, head): chunked causal softmax(q k^T / 8) @ v with ones-append
      row-sum trick -> y^T [64, 2048]
    AllGather y^T over cores -> [1024, 2048] per batch (full head dim)
    out^T slice = w_proj_slice^T @ y^T_full  [128, 4096] bf16 -> host
  Host: assemble out columns, + b_proj, cast f32.
"""

import sys

if "/opt/trn_rl_repo" not in sys.path:
    sys.path.insert(0, "/opt/trn_rl_repo")

from contextlib import ExitStack

import numpy as np

import concourse.bacc as bacc
import concourse.mybir as mybir
import concourse.tile as tile
from concourse.masks import make_upper_triangular

N_CORES = 8
B = 2
T = 2048          # tokens per batch
TT = B * T        # 4096
C = 1024
HL = 2            # local heads per core
D = 64            # head dim
QK = 2 * HL * D   # 256 q+k channels per core
V = HL * D        # 128 v channels per core
P = 128
NT = T // P       # 16 token tiles per batch
NCC = C // P      # 8 contraction chunks
SCALE = D ** -0.5
f32 = mybir.dt.float32
bf16 = mybir.dt.bfloat16
AF = mybir.ActivationFunctionType
RG = [list(range(N_CORES))]


def _slices_aligned(start, end):
    """[start, end) split on the 512 grid (PSUM-bank-aligned outputs)."""
    out = []
    n0 = start
    while n0 < end:
        n1 = min(end, (n0 // 512 + 1) * 512)
        out.append((n0, n1))
        n0 = n1
    return out


def build():
    nc = bacc.Bacc("TRN2", target_bir_lowering=False, debug=False,
                   num_devices=N_CORES)

    xt_ap = nc.dram_tensor("xt", [P, TT], bf16, kind="ExternalInput").ap()
    w_qkv_ap = nc.dram_tensor("w_qkv", [C, 3 * V], bf16,
                              kind="ExternalInput").ap()
    b_qk_ap = nc.dram_tensor("b_qk", [P, 2], f32, kind="ExternalInput").ap()
    b_v_ap = nc.dram_tensor("b_v", [1, V], bf16, kind="ExternalInput").ap()
    w_proj_ap = nc.dram_tensor("w_proj", [C, P], bf16,
                               kind="ExternalInput").ap()
    out_ap = nc.dram_tensor("outT", [P, TT], bf16, kind="ExternalOutput").ap()

    with tile.TileContext(nc) as tc, ExitStack() as ctx:
        dram = ctx.enter_context(tc.tile_pool(name="dram", bufs=1,
                                              space="DRAM"))
        # all collectives split into 1024-token halves so each AllGather
        # pipelines behind compute: x halves feed phase A incrementally,
        # y halves launch as soon as their q-half of attention finishes.
        TH = T // 2
        agx_in = [dram.tile([P, TH], bf16, tag=f"agxi{i}", name=f"agxi{i}")
                  for i in range(2 * B)]
        agx_out = [dram.tile([C, TH], bf16, tag=f"agxo{i}", name=f"agxo{i}",
                             addr_space="Shared")
                   for i in range(2 * B)]
        agy_in = [dram.tile([P, TH], bf16, tag=f"agyi{i}", name=f"agyi{i}")
                  for i in range(2 * B)]
        agy_out = [dram.tile([C, TH], bf16, tag=f"agyo{i}", name=f"agyo{i}",
                             addr_space="Shared")
                   for i in range(2 * B)]

        # stage x^T halves into collective bounce buffers, kick AllGathers
        for i in range(2 * B):
            nc.gpsimd.dma_start(agx_in[i][:], xt_ap[:, i * TH:(i + 1) * TH])
        for i in range(2 * B):
            nc.gpsimd.collective_compute(
                "AllGather", mybir.AluOpType.bypass, replica_groups=RG,
                ins=[agx_in[i].opt()], outs=[agx_out[i].opt()])

        const_pool = ctx.enter_context(tc.tile_pool(name="const", bufs=1))
        # keep element [j, i] iff j <= i  (upper triangular incl diag)
        mask01 = const_pool.tile([P, P], bf16, tag="mask01", name="mask01")
        make_upper_triangular(nc, mask01[:], val=1.0, diag=True)
        ones_row = const_pool.tile([1, P], bf16, tag="ones", name="ones")
        nc.vector.memset(ones_row[:], 1.0)
        ones_f32 = const_pool.tile([1, D], f32, tag="onesf", name="onesf")
        nc.vector.memset(ones_f32[:], 1.0)
        ones_col = const_pool.tile([P, HL], bf16, tag="onesc", name="onesc")
        nc.vector.memset(ones_col[:], 1.0)
        bqk_t = const_pool.tile([P, 2], f32, tag="bqk", name="bqk")
        nc.sync.dma_start(bqk_t[:], b_qk_ap)
        bv_row = const_pool.tile([1, V], bf16, tag="bv", name="bv")
        nc.sync.dma_start(bv_row[:], b_v_ap)

        # weights resident in SBUF
        w_pool = ctx.enter_context(tc.tile_pool(name="wp", bufs=1))
        wqkv = [w_pool.tile([P, 3 * V], bf16, tag=f"wqkv{c}", name=f"wqkv{c}")
                for c in range(NCC)]
        wproj = [w_pool.tile([P, P], bf16, tag=f"wpr{c}", name=f"wpr{c}")
                 for c in range(NCC)]
        wqkv_view = w_qkv_ap.rearrange("(c p) n -> c p n", p=P)
        wproj_view = w_proj_ap.rearrange("(c p) n -> c p n", p=P)
        for c in range(NCC):
            nc.sync.dma_start(wqkv[c][:], wqkv_view[c])
            nc.sync.dma_start(wproj[c][:], wproj_view[c])

        # persistent intermediates
        qk_pool = ctx.enter_context(tc.tile_pool(name="qkp", bufs=1))
        qk_sb = [qk_pool.tile([P, TT], bf16, tag=f"qk{m}", name=f"qk{m}")
                 for m in range(2)]
        v_pool = ctx.enter_context(tc.tile_pool(name="vp", bufs=1))
        v_sb = [v_pool.tile([P, HL * (D + 1)], bf16, tag=f"v{t}",
                            name=f"v{t}") for t in range(B * NT)]
        # one yT tile per (batch, q-half) so the y AllGather for a half has
        # an exact dependency on just that half's writes
        yT_pool = ctx.enter_context(tc.tile_pool(name="yTp", bufs=1))
        yT_sb = [yT_pool.tile([P, T // 2], bf16, tag=f"yT{i}", name=f"yT{i}")
                 for i in range(2 * B)]

        # ---------------- Phase A: qk^T and v per batch ----------------
        def phase_a(b):
            for ts in range(4):
                s0 = ts * 512
                xsrc = agx_out[2 * b + s0 // TH]
                c0 = s0 % TH
                xsb = []
                for c in range(NCC):
                    xt_t = xsb_pool.tile([P, 512], bf16, tag=f"x{c}",
                                         name="xt_t")
                    nc.sync.dma_start(
                        xt_t[:], xsrc[c * P:(c + 1) * P, c0:c0 + 512])
                    xsb.append(xt_t)
                for m in range(2):
                    ps = qkps_pool.tile([P, 512], f32, tag="qkps", name="ps")
                    for c in range(NCC):
                        nc.tensor.matmul(
                            ps[:], lhsT=wqkv[c][:, m * P:(m + 1) * P],
                            rhs=xsb[c][:], start=(c == 0), stop=(c == NCC - 1))
                    nc.scalar.activation(
                        qk_sb[m][:, b * T + s0:b * T + s0 + 512], ps[:],
                        AF.Identity, bias=bqk_t[:, m:m + 1], scale=1.0)
                for tt in range(4):
                    vp = vps_pool.tile([P, V], f32, tag="vps", name="vp")
                    for c in range(NCC):
                        nc.tensor.matmul(
                            vp[:], lhsT=xsb[c][:, tt * P:(tt + 1) * P],
                            rhs=wqkv[c][:, 2 * P:3 * P],
                            start=(c == 0), stop=False)
                    # bias as rank-1 update: ones[T,1] @ b_v[1,V]
                    nc.tensor.matmul(
                        vp[:], lhsT=ones_row[0:1, 0:P], rhs=bv_row[:],
                        start=False, stop=True)
                    v3 = v_sb[b * NT + ts * 4 + tt][:].rearrange(
                        "p (h e) -> p h e", e=D + 1)
                    nc.vector.tensor_copy(
                        v3[:, :, 0:D],
                        vp[:].rearrange("p (h d) -> p h d", d=D))
                    nc.vector.tensor_copy(
                        v3[:, :, D:D + 1],
                        ones_col[:].rearrange("p (h o) -> p h o", o=1))

        # ---------------- Phase B: attention per batch, heads paired ----
        # The two local heads' s^T matmuls have K=64 contraction; head 0
        # lives on array rows 0-63 (base_partition 0), head 1 on rows
        # 64-127 (base_partition 64, auto tile_position) -- emitted
        # back-to-back they run CONCURRENTLY on the PE's row groups.
        # PSUM budget forces q-halving: per q-half, yext[h] is [65, 1024]
        # (2 banks) x 2 heads + 4 s-psum bufs = 8 banks.
        def phase_b_pair(b, qh):
            qbase = qh * 1024
            qend = qbase + 1024
            cmax = 8 * (qh + 1)
            qT = [qk_sb[0][h * D:(h + 1) * D, b * T:(b + 1) * T]
                  for h in range(HL)]
            kT = [qk_sb[1][h * D:(h + 1) * D, b * T:(b + 1) * T]
                  for h in range(HL)]
            yext = [yext_pool.tile([D + 1, 1024], f32, tag=f"yext{h}",
                                   name=f"yext{h}") for h in range(HL)]

            def emit_st_exp(c):
                """Paired s^T matmuls + exp for chunk c -> pT tiles."""
                k0 = c * P
                q0 = max(k0, qbase)
                pT = [pt_pool.tile([P, 1024], bf16, tag=f"pt{h}", name="pT")
                      for h in range(HL)]
                for (n0, n1) in _slices_aligned(q0, qend):
                    for h in range(HL):
                        sp = sps_pool.tile([P, 512], f32, tag="sps",
                                           name="sp")
                        nc.tensor.matmul(
                            sp[:, 0:n1 - n0], lhsT=kT[h][:, k0:k0 + P],
                            rhs=qT[h][:, n0:n1], start=True, stop=True)
                        nc.scalar.activation(
                            pT[h][:, n0 - qbase:n1 - qbase], sp[:, 0:n1 - n0],
                            AF.Exp, bias=0.0, scale=SCALE)
                if k0 >= qbase:
                    # causal mask inside the diagonal block
                    for h in range(HL):
                        nc.vector.tensor_mul(
                            pT[h][:, k0 - qbase:k0 - qbase + P],
                            pT[h][:, k0 - qbase:k0 - qbase + P], mask01[:])
                return pT

            def emit_pv(c, pT):
                k0 = c * P
                q0 = max(k0, qbase)
                for h in range(HL):
                    for (n0, n1) in _slices_aligned(q0, qend):
                        nc.tensor.matmul(
                            yext[h][:, n0 - qbase:n1 - qbase],
                            lhsT=v_sb[b * NT + c][:,
                                                  h * (D + 1):(h + 1) * (D + 1)],
                            rhs=pT[h][:, n0 - qbase:n1 - qbase],
                            start=(c == 0), stop=(c == cmax - 1),
                            skip_group_check=True)

            # software pipeline: emit s^T(c+1) before pv(c) so the PE
            # fills the exp(c) latency with the next chunk's matmuls
            pT_prev = emit_st_exp(0)
            for c in range(1, cmax):
                pT_cur = emit_st_exp(c)
                emit_pv(c - 1, pT_prev)
                pT_prev = pT_cur
            emit_pv(cmax - 1, pT_prev)
            # normalize rows by l (last partition row of yext) and
            # store into yT in [d, T] layout
            for h in range(HL):
                po = h * D
                for g2 in range(2):
                    s0, s1 = g2 * 512, (g2 + 1) * 512
                    rr = rr_pool.tile([1, 512], f32, tag="rr", name="rr")
                    nc.vector.reciprocal(rr[:], yext[h][D:D + 1, s0:s1])
                    bpt = sps_pool.tile([P, 512], f32, tag="sps", name="bp")
                    bp = bpt[0:D, :]
                    nc.tensor.matmul(bp, lhsT=ones_f32[:], rhs=rr[:],
                                     start=True, stop=True)
                    rb = rbc_pool.tile([D, 512], f32, tag="rbc", name="rb")
                    nc.vector.tensor_copy(rb[:], bp)
                    with nc.allow_low_precision(
                            reason="bf16 y store; 2e-2 rel-err budget"):
                        nc.vector.tensor_mul(
                            yT_sb[2 * b + qh][po:po + D, s0:s1],
                            yext[h][0:D, s0:s1], rb[:])

        # ---------------- Phase C: output projection per batch ----------
        def launch_agy(i):
            nc.gpsimd.dma_start(agy_in[i][:], yT_sb[i][:])
            nc.gpsimd.collective_compute(
                "AllGather", mybir.AluOpType.bypass, replica_groups=RG,
                ins=[agy_in[i].opt()], outs=[agy_out[i].opt()])

        def phase_c(b):
            for ts in range(4):
                s0 = ts * 512
                ysrc = agy_out[2 * b + s0 // TH]
                c0 = s0 % TH
                pp = pp_pool.tile([P, 512], f32, tag="pp", name="pp")
                for c in range(NCC):
                    ysb = ysb_pool.tile([P, 512], bf16, tag="y", name="ysb")
                    nc.sync.dma_start(
                        ysb[:], ysrc[c * P:(c + 1) * P, c0:c0 + 512])
                    nc.tensor.matmul(pp[:], lhsT=wproj[c][:], rhs=ysb[:],
                                     start=(c == 0), stop=(c == NCC - 1))
                ob = osb_pool.tile([P, 512], bf16, tag="osb", name="ob")
                nc.scalar.copy(ob[:], pp[:])
                nc.sync.dma_start(out_ap[:, b * T + s0:b * T + s0 + 512],
                                  ob[:])

        with ExitStack() as actx:
            xsb_pool = actx.enter_context(tc.tile_pool(name="xsb", bufs=2))
            qkps_pool = actx.enter_context(
                tc.tile_pool(name="qkps", bufs=3, space="PSUM"))
            vps_pool = actx.enter_context(
                tc.tile_pool(name="vps", bufs=4, space="PSUM"))
            phase_a(0)
            phase_a(1)
        with ExitStack() as bctx:
            pt_pool = bctx.enter_context(tc.tile_pool(name="pt", bufs=2))
            rr_pool = bctx.enter_context(tc.tile_pool(name="rr", bufs=2))
            rbc_pool = bctx.enter_context(tc.tile_pool(name="rbc", bufs=2))
            sps_pool = bctx.enter_context(
                tc.tile_pool(name="sps", bufs=4, space="PSUM"))
            yext_pool = bctx.enter_context(
                tc.tile_pool(name="yext", bufs=1, space="PSUM"))
            # each q-half's y AllGather launches as soon as that half's
            # attention is normalized, hiding the gather behind the next
            # half's / batch's compute
            phase_b_pair(0, 0)
            launch_agy(0)
            phase_b_pair(0, 1)
            launch_agy(1)
            phase_b_pair(1, 0)
            launch_agy(2)
            phase_b_pair(1, 1)
            launch_agy(3)
        ysb_pool = ctx.enter_context(tc.tile_pool(name="ysb", bufs=4))
        osb_pool = ctx.enter_context(tc.tile_pool(name="osb", bufs=3))
        pp_pool = ctx.enter_context(
            tc.tile_pool(name="pp", bufs=2, space="PSUM"))
        phase_c(0)             # overlaps agy(1)
        phase_c(1)

    nc.compile()
    return nc


_NC = None


def _get_nc():
    global _NC
    if _NC is None:
        _NC = build()
    return _NC


def make_in_maps(x, W_qkv, b_qkv, W_proj):
    """Per-core input dicts (host-side sharding, bf16)."""
    import ml_dtypes

    bf = ml_dtypes.bfloat16
    x = np.asarray(x, dtype=np.float32)
    W_qkv = np.asarray(W_qkv, dtype=np.float32)
    b_qkv = np.asarray(b_qkv, dtype=np.float32)
    W_proj = np.asarray(W_proj, dtype=np.float32)
    # x^T with both batches side by side: [C, B*T]
    xT = np.concatenate([x[0].T, x[1].T], axis=1).astype(bf)
    Wb = W_qkv.astype(bf)
    Wpb = W_proj.astype(bf)
    in_maps = []
    for c in range(N_CORES):
        q0 = P * c
        k0 = C + P * c
        v0 = 2 * C + P * c
        in_maps.append({
            "xt": np.ascontiguousarray(xT[P * c:P * (c + 1), :]),
            "w_qkv": np.ascontiguousarray(
                np.concatenate([Wb[:, q0:q0 + P], Wb[:, k0:k0 + P],
                                Wb[:, v0:v0 + P]], axis=1)),
            "b_qk": np.ascontiguousarray(
                np.stack([b_qkv[q0:q0 + P], b_qkv[k0:k0 + P]], axis=1)),
            "b_v": np.ascontiguousarray(
                b_qkv[v0:v0 + P].astype(bf).reshape(1, V)),
            "w_proj": np.ascontiguousarray(Wpb[:, P * c:P * (c + 1)]),
        })
    return in_maps


def combine(results, b_proj):
    """Host-side unshard: concat per-core out^T column slices, + bias."""
    b_proj = np.asarray(b_proj, dtype=np.float32)
    # results[c]["outT"]: [128, 4096] bf16 -> out[:, :, 128c:128c+128]
    cols = [results[c]["outT"].astype(np.float32) for c in range(N_CORES)]
    full = np.concatenate(cols, axis=0)            # [1024, 4096]
    out = full.T.reshape(B, T, C) + b_proj
    return np.ascontiguousarray(out)


def kernel(x, W_qkv, b_qkv, W_proj, b_proj):
    from concourse.bass_utils import run_bass_kernel_spmd

    nc = _get_nc()
    in_maps = make_in_maps(x, W_qkv, b_qkv, W_proj)
    res = run_bass_kernel_spmd(nc, in_maps, list(range(N_CORES)))
    return combine(res.results, b_proj)
